# revision 15
# baseline (speedup 1.0000x reference)
"""DTA model (3-layer GATv2 + 2-layer transformer encoder + MLP heads) on 8 trn2 cores.

Sharding: nodes -> 8 contiguous ranges (padded to 50176 = 8*49*128); edge dst-tiles
aligned to node ranges; protein batch -> 16/core; params replicated.
GAT edge stage: per-dst-tile mask matmuls (scatter/expand on PE), indirect-DMA
gathers from a shared xl table, softmax normalized on edges via reciprocal-den expand.
Transformer: transposed (feature-major) layout, bf16 matmuls, f32 LN statistics.
"""

import numpy as np

import concourse.bass as bass
import concourse.bacc as bacc
import concourse.mybir as mybir
import concourse.tile as tile
from concourse.bass_utils import run_bass_kernel_spmd
from concourse.masks import make_identity

F32 = mybir.dt.float32
F32R = mybir.dt.float32r
BF16 = mybir.dt.bfloat16
I32 = mybir.dt.int32
AX = mybir.AxisListType
OP = mybir.AluOpType
ACTF = mybir.ActivationFunctionType

N = 50000
NP = 50176
PC = 6272
T = 49
NCORES = 8
B = 128
L = 512
VOCAB = 26
VP = 32
H = 4
C = 128
HC = 512
D = 128
FF = 2048
NF = FF // D
NL = 2
EPS = 1e-5
LP = 16

DEBUG = False


# ----------------------------------------------------------------------------
# Host-side preprocessing
# ----------------------------------------------------------------------------

def _prep(x, edge_index, batch, target, p):
    src = np.asarray(edge_index[0])
    dst = np.asarray(edge_index[1])
    loops = np.arange(N, dtype=src.dtype)
    src = np.concatenate([src, loops])
    dst = np.concatenate([dst, loops])
    order = np.argsort(dst, kind="stable")
    src = src[order]
    dst = dst[order]

    ntiles = NP // 128
    tile_of = dst // 128
    counts = np.bincount(tile_of, minlength=ntiles)
    starts = np.concatenate([[0], np.cumsum(counts)])

    meta = [max(1, int(-(-max(counts[k * T + tl] for k in range(NCORES)) // 128)))
            for tl in range(T)]
    CH = sum(meta)

    core_srcs, core_maskT, core_maskde = [], [], []
    core_xlids, core_gmask = [], []
    bpad = np.full(NP, 255, dtype=np.int64)
    bpad[:N] = np.asarray(batch)
    for k in range(NCORES):
        srcs = np.zeros((128, CH), np.int32)
        mT = np.zeros((128, CH * 128), np.float32)
        mde = np.zeros((128, CH * 128), np.float32)
        choff = 0
        for tl in range(T):
            K = meta[tl]
            gt = k * T + tl
            s0, s1 = starts[gt], starts[gt + 1]
            e = s1 - s0
            cap = K * 128
            s_pad = np.zeros(cap, dtype=np.int64)
            d_pad = np.full(cap, 255, dtype=np.int64)
            s_pad[:e] = src[s0:s1]
            d_pad[:e] = dst[s0:s1] - gt * 128
            srcs[:, choff:choff + K] = s_pad.reshape(K, 128).T
            dd = d_pad.reshape(K, 128)
            m1 = (dd[:, :, None] == np.arange(128)[None, None, :])
            mT[:, choff * 128:(choff + K) * 128] = \
                np.transpose(m1, (1, 0, 2)).reshape(128, K * 128)
            m2 = (np.arange(128)[:, None, None] == dd[None, :, :])
            mde[:, choff * 128:(choff + K) * 128] = m2.reshape(128, K * 128)
            choff += K
        core_srcs.append(srcs)
        core_maskT.append(mT)
        core_maskde.append(mde)
        ids = (np.arange(PC, dtype=np.int32) + k * PC).reshape(T, 128).T
        core_xlids.append(np.ascontiguousarray(ids))
        bk = bpad[k * PC:(k + 1) * PC].reshape(T, 128)
        gm = (bk[:, :, None] == np.arange(B)[None, None, :])
        core_gmask.append(np.transpose(gm, (1, 0, 2)).reshape(128, T * B)
                          .astype(np.float32))

    cnt = np.bincount(np.asarray(batch), minlength=B).astype(np.float32)
    inv_cnt = (1.0 / np.maximum(cnt, 1.0)).reshape(B, 1).astype(np.float32)

    xT0 = np.zeros((80, NP), np.float32)
    xT0[:78, :N] = np.asarray(x).T
    xT0[78, :] = 1.0

    tg = np.asarray(target).reshape(-1)
    oh = np.zeros((VP, B * L), np.float32)
    oh[tg, np.arange(B * L)] = 1.0
    emb_pad = np.zeros((VP, D), np.float32)
    emb_pad[:VOCAB] = np.asarray(p["embed"])

    import ml_dtypes
    bf = lambda a: np.ascontiguousarray(np.asarray(a, dtype=ml_dtypes.bfloat16))
    f32 = lambda a: np.ascontiguousarray(np.asarray(a, dtype=np.float32))

    shared = {}
    for l in range(3):
        wl = np.asarray(p[f"g{l}_wl"]); bl = np.asarray(p[f"g{l}_bl"])
        wr = np.asarray(p[f"g{l}_wr"]); br = np.asarray(p[f"g{l}_br"])
        if l == 0:
            wlT = np.zeros((80, HC), np.float32); wlT[:78] = wl.T; wlT[78] = bl
            wrT = np.zeros((80, HC), np.float32); wrT[:78] = wr.T; wrT[78] = br
            shared["wlT0"] = bf(wlT); shared["wrT0"] = bf(wrT)
        else:
            shared[f"wlT{l}"] = bf(wl.T); shared[f"wrT{l}"] = bf(wr.T)
            shared[f"blrow{l}"] = bf(bl.reshape(1, HC))
            shared[f"brrow{l}"] = bf(br.reshape(1, HC))
        att = np.asarray(p[f"g{l}_att"]).reshape(HC)
        shared[f"attrep{l}"] = bf(np.broadcast_to(att, (128, HC)))
        shared[f"biasrep{l}"] = bf(np.broadcast_to(np.asarray(p[f"g{l}_b"]), (128, C)))

    for l in range(NL):
        shared[f"wqkvT{l}"] = bf(np.asarray(p[f"t{l}_wqkv"]).T)
        shared[f"bqkv{l}"] = f32(np.asarray(p[f"t{l}_bqkv"]).reshape(3, D).T)
        shared[f"bvrow{l}"] = bf(np.asarray(p[f"t{l}_bqkv"])[2 * D:].reshape(1, D))
        shared[f"woT{l}"] = bf(np.asarray(p[f"t{l}_wo"]).T)
        shared[f"bo{l}"] = f32(np.asarray(p[f"t{l}_bo"]).reshape(D, 1))
        shared[f"w1T{l}"] = bf(np.asarray(p[f"t{l}_w1"]).T)
        shared[f"b1{l}"] = f32(np.asarray(p[f"t{l}_b1"]).reshape(NF, D).T)
        w2T = np.asarray(p[f"t{l}_w2"]).T
        shared[f"w2T{l}"] = bf(w2T.reshape(NF, D, D).transpose(1, 0, 2)
                               .reshape(D, NF * D))
        shared[f"b2{l}"] = f32(np.asarray(p[f"t{l}_b2"]).reshape(D, 1))
        for nm in ("ln1g", "ln1b", "ln2g", "ln2b"):
            shared[f"{nm}{l}"] = f32(np.asarray(p[f"t{l}_{nm}"]).reshape(D, 1))

    shared["emb_pad"] = f32(emb_pad)
    shared["fcwT"] = f32(np.asarray(p["fc_w"]).T / L)
    shared["fcb"] = f32(np.asarray(p["fc_b"]).reshape(D, 1))
    r1T = np.asarray(p["r1_w"]).T
    shared["r1wT"] = f32(r1T.reshape(2, D, 512).transpose(1, 0, 2).reshape(D, 1024))
    shared["r1b"] = f32(np.asarray(p["r1_b"]).reshape(4, 128).T)
    r2T = np.asarray(p["r2_w"]).T
    shared["r2wT"] = f32(r2T.reshape(4, D, D).transpose(1, 0, 2).reshape(D, 4 * D))
    shared["r2b"] = f32(np.asarray(p["r2_b"]).reshape(D, 1))
    shared["r3wT"] = f32(np.asarray(p["r3_w"]).T)
    shared["r3b"] = f32(np.asarray(p["r3_b"]).reshape(1, 1))
    for nm in ("dp1", "dp2", "pp1", "pp2"):
        shared[f"{nm}T"] = f32(np.asarray(p[f"{nm}_w"]).T)
        shared[f"{nm}b"] = f32(np.asarray(p[f"{nm}_b"]).reshape(-1, 1))
    shared["inv_cnt"] = inv_cnt

    in_maps = []
    for k in range(NCORES):
        m = dict(shared)
        m["xT0"] = bf(xT0[:, k * PC:(k + 1) * PC])
        m["srcids"] = core_srcs[k]
        m["maskT"] = bf(core_maskT[k])
        m["maskde"] = bf(core_maskde[k])
        m["xlids"] = core_xlids[k]
        m["gmask"] = bf(core_gmask[k])
        m["onehotT"] = f32(oh[:, k * LP * L:(k + 1) * LP * L])
        sel = np.zeros((B, LP), np.float32)
        sel[np.arange(k * LP, (k + 1) * LP), np.arange(LP)] = 1.0
        m["sel"] = sel
        lnsel = np.zeros((LP, LP * 128), np.float32)
        for pp in range(LP):
            lnsel[pp, pp * 128:(pp + 1) * 128] = 1.0
        m["lnsel"] = bf(lnsel)
        in_maps.append(m)
    return in_maps, meta, CH


# ----------------------------------------------------------------------------
# Program builder
# ----------------------------------------------------------------------------

def _build(meta, CH):
    nc = bacc.Bacc("TRN2", target_bir_lowering=False, debug=False,
                   num_devices=NCORES, num_swdge_queues=4)

    din = lambda nm, sh, dt: nc.declare_dram_parameter(nm, list(sh), dt, isOutput=False)
    dout = lambda nm, sh, dt: nc.declare_dram_parameter(nm, list(sh), dt, isOutput=True)

    xT0 = din("xT0", [80, PC], BF16)
    srcids = din("srcids", [128, CH], I32)
    maskT_in = din("maskT", [128, CH * 128], BF16)
    maskde_in = din("maskde", [128, CH * 128], BF16)
    xlids = din("xlids", [128, T], I32)
    gmask_in = din("gmask", [128, T * B], BF16)
    onehotT = din("onehotT", [VP, LP * L], F32)
    sel_in = din("sel", [B, LP], F32)
    lnsel_in = din("lnsel", [LP, LP * 128], BF16)
    inv_cnt = din("inv_cnt", [B, 1], F32)

    bf_names = []
    for l in range(3):
        dims = 80 if l == 0 else 128
        bf_names += [(f"wlT{l}", [dims, HC]), (f"wrT{l}", [dims, HC]),
                     (f"attrep{l}", [128, HC]), (f"biasrep{l}", [128, C])]
        if l > 0:
            bf_names += [(f"blrow{l}", [1, HC]), (f"brrow{l}", [1, HC])]
    for l in range(NL):
        bf_names += [(f"wqkvT{l}", [D, 3 * D]), (f"bvrow{l}", [1, D]),
                     (f"woT{l}", [D, D]), (f"w1T{l}", [D, FF]),
                     (f"w2T{l}", [D, FF])]
    bf_in = {nm: din(nm, sh, BF16) for nm, sh in bf_names}

    f_names = []
    for l in range(NL):
        f_names += [(f"bqkv{l}", [D, 3]), (f"bo{l}", [D, 1]),
                    (f"b1{l}", [D, NF]), (f"b2{l}", [D, 1]),
                    (f"ln1g{l}", [D, 1]), (f"ln1b{l}", [D, 1]),
                    (f"ln2g{l}", [D, 1]), (f"ln2b{l}", [D, 1])]
    f_names += [("emb_pad", [VP, D]), ("fcwT", [D, D]), ("fcb", [D, 1]),
                ("r1wT", [D, 1024]), ("r1b", [128, 4]),
                ("r2wT", [D, 4 * D]), ("r2b", [D, 1]),
                ("r3wT", [D, 1]), ("r3b", [1, 1]),
                ("dp1T", [D, 64]), ("dp1b", [64, 1]), ("dp2T", [64, 64]),
                ("dp2b", [64, 1]), ("pp1T", [D, 64]), ("pp1b", [64, 1]),
                ("pp2T", [64, 64]), ("pp2b", [64, 1])]
    f_in = {nm: din(nm, sh, F32) for nm, sh in f_names}

    predT_o = dout("predT", [1, LP], F32)
    zdT_o = dout("zdT", [64, LP], F32)
    zpT_o = dout("zpT", [64, LP], F32)
    dbg = {}
    if DEBUG:
        dbg["h1"] = dout("dbg_h1", [128, C], F32)
        dbg["demb"] = dout("dbg_demb", [B, C], F32)
        dbg["pemb"] = dout("dbg_pemb", [D, LP], F32)
        dbg["xl0"] = dout("dbg_xl0", [128, HC], F32)

    xl_sh = [nc.dram_tensor(f"xl_sh{i}", [NP + 2, HC], BF16, addr_space="Shared")
             for i in range(2)]
    xr_loc = nc.dram_tensor("xr_loc", [PC, HC], BF16)
    dpool_loc = nc.dram_tensor("dpool_loc", [B, C], F32)
    dpool_sh = nc.dram_tensor("dpool_sh", [B, C], F32, addr_space="Shared")
    bar_loc = nc.dram_tensor("bar_loc", [3, 128], BF16)

    ctxs = []

    def pool_enter(p):
        ctxs.append(p)
        return p.__enter__()

    with tile.TileContext(nc) as tc, \
         nc.allow_low_precision(reason="bf16/f32r intermediates are intentional"):
        cst = pool_enter(tc.tile_pool(name="cst", bufs=1))
        wp = pool_enter(tc.tile_pool(name="wts", bufs=1))
        sb = pool_enter(tc.tile_pool(name="sb", bufs=2))       # edge transients
        sbt = pool_enter(tc.tile_pool(name="sbt", bufs=1))     # TF transients
        sbg = pool_enter(tc.tile_pool(name="sbg", bufs=2))     # big edge tiles
        sb2 = pool_enter(tc.tile_pool(name="sb2", bufs=2))
        ps = pool_enter(tc.tile_pool(name="ps", bufs=2, space="PSUM"))
        psl = pool_enter(tc.tile_pool(name="psl", bufs=1, space="PSUM"))
        pse = pool_enter(tc.tile_pool(name="pse", bufs=2, space="PSUM"))

        MM = dict(tag="mm512")

        ident = cst.tile([128, 128], BF16, tag="ident")
        make_identity(nc, ident[:])
        ones1 = cst.tile([1, 128], BF16, tag="ones1")
        nc.vector.memset(ones1[:], 1.0)
        ones128b = cst.tile([128, 1], BF16, tag="ones128b")
        nc.vector.memset(ones128b[:], 1.0)

        xT0_sb = cst.tile([80, PC], BF16, tag="xT0sb")
        nc.sync.dma_start(out=xT0_sb[:], in_=xT0[:])
        hT_a = cst.tile([128, PC], BF16, tag="hT_a")
        hT_b = cst.tile([128, PC], BF16, tag="hT_b")
        nc.vector.memset(hT_a[:, PC - 256:], 0.0)
        nc.vector.memset(hT_b[:, PC - 256:], 0.0)
        xTs = cst.tile([128, L * LP], BF16, tag="xTs")
        S_sb = cst.tile([LP, L], F32, tag="S_sb")
        Q_sb = cst.tile([LP, L], F32, tag="Q_sb")
        eps_col = cst.tile([LP, 1], F32, tag="eps_col")
        nc.vector.memset(eps_col[:], EPS)

        bw = {}
        for nm, _ in bf_names:
            t_in = bf_in[nm]
            bw[nm] = wp.tile(list(t_in.shape), BF16, tag=nm, name=nm)
            nc.sync.dma_start(out=bw[nm][:], in_=t_in[:])
        tw = {}
        for nm, _ in f_names:
            t_in = f_in[nm]
            tw[nm] = wp.tile(list(t_in.shape), F32R, tag=nm, name="t_" + nm)
            nc.sync.dma_start(out=tw[nm][:], in_=t_in[:].bitcast(F32R))
        selsb = wp.tile([B, LP], F32R, tag="selsb")
        nc.sync.dma_start(out=selsb[:], in_=sel_in[:].bitcast(F32R))
        lnsel = wp.tile([LP, LP * 128], BF16, tag="lnsel")
        nc.sync.dma_start(out=lnsel[:], in_=lnsel_in[:])
        invc = wp.tile([B, 1], F32, tag="invc")
        nc.sync.dma_start(out=invc[:], in_=inv_cnt[:])
        xlids_sb = wp.tile([128, T], I32, tag="xlids_sb")
        nc.sync.dma_start(out=xlids_sb[:], in_=xlids[:])

        def fb(ap):
            return ap.bitcast(F32)

        # =============================== GAT =================================
        def node_stage(l, hT_src, dim_in):
            for tl in range(T):
                lhsT = hT_src[:dim_in, tl * 128:(tl + 1) * 128]
                for which in ("xl", "xr"):
                    wkey = f"w{'l' if which == 'xl' else 'r'}T{l}"
                    bkey = f"b{'l' if which == 'xl' else 'r'}row{l}"
                    pt = ps.tile([128, HC], F32, **MM)
                    nc.tensor.matmul(out=pt[:], lhsT=lhsT, rhs=bw[wkey][:],
                                     start=True, stop=(l == 0))
                    if l > 0:
                        nc.tensor.matmul(out=pt[:], lhsT=ones1[:],
                                         rhs=bw[bkey][:], start=False, stop=True)
                    st = sb.tile([128, HC], BF16, tag="node_sb")
                    nc.scalar.activation(out=st[:], in_=pt[:], func=ACTF.Copy)
                    if which == "xl":
                        nc.gpsimd.indirect_dma_start(
                            out=xl_sh[l % 2][:NP, :], in_=st[:],
                            out_offset=bass.IndirectOffsetOnAxis(
                                ap=xlids_sb[:, tl:tl + 1], axis=0),
                            in_offset=None)
                        if DEBUG and l == 0 and tl == 0:
                            dpt = sb.tile([128, HC], F32, tag="dbgxl")
                            nc.vector.tensor_copy(out=dpt[:], in_=st[:])
                            nc.sync.dma_start(out=dbg["xl0"][:], in_=dpt[:])
                    else:
                        nc.sync.dma_start(
                            out=xr_loc[tl * 128:(tl + 1) * 128, :], in_=st[:])

        def barrier(l):
            buf = xl_sh[l % 2]
            tok = sb.tile([1, 128], BF16, tag="bar_tok")
            nc.sync.dma_start(out=tok[:], in_=buf[NP:NP + 1, :128])
            nc.sync.dma_start(out=bar_loc[l:l + 1, :], in_=tok[:])
            nc.gpsimd.collective_compute(
                "AllReduce", OP.add,
                ins=[bar_loc[l:l + 1, :]], outs=[buf[NP + 1:NP + 2, :128]],
                replica_groups=[list(range(NCORES))])

        def edge_stage(l, hT_dst, pool_psum):
            srcs_all = sb2.tile([128, CH], I32, tag="srcs_all")
            nc.sync.dma_start(out=srcs_all[:], in_=srcids[:])
            choff = 0
            for tl in range(T):
                K = meta[tl]
                xrt = sb.tile([128, HC], BF16, tag="xrt")
                nc.sync.dma_start(out=xrt[:],
                                  in_=xr_loc[tl * 128:(tl + 1) * 128, :])
                mT = sb.tile([128, K * 128], BF16, tag="mT")
                nc.sync.dma_start(
                    out=mT[:], in_=maskT_in[:, choff * 128:(choff + K) * 128])
                mde = sb.tile([128, K * 128], BF16, tag="mde")
                nc.sync.dma_start(
                    out=mde[:], in_=maskde_in[:, choff * 128:(choff + K) * 128])

                xlg = sbg.tile([128, K * HC], BF16, tag="xlg")
                ex_all = sb.tile([128, K * H], BF16, tag="ex_all")
                for c in range(K):
                    nc.gpsimd.indirect_dma_start(
                        out=xlg[:, c * HC:(c + 1) * HC], out_offset=None,
                        in_=xl_sh[l % 2][:, :],
                        in_offset=bass.IndirectOffsetOnAxis(
                            ap=srcs_all[:, choff + c:choff + c + 1], axis=0))
                    pm = ps.tile([128, HC], F32, **MM)
                    nc.tensor.matmul(out=pm[:], lhsT=mde[:, c * 128:(c + 1) * 128],
                                     rhs=xrt[:], start=True, stop=False)
                    nc.tensor.matmul(out=pm[:], lhsT=ident[:],
                                     rhs=xlg[:, c * HC:(c + 1) * HC],
                                     start=False, stop=True)
                    lr = sb.tile([128, HC], BF16, tag="lr")
                    nc.scalar.activation(out=lr[:], in_=pm[:], func=ACTF.Prelu,
                                         alpha=0.2)
                    tm = sb.tile([128, HC], BF16, tag="tm")
                    nc.vector.tensor_tensor(out=tm[:], in0=lr[:],
                                            in1=bw[f"attrep{l}"][:], op=OP.mult)
                    al = sb.tile([128, H], F32, tag="al")
                    nc.vector.tensor_reduce(
                        out=al[:], in_=tm[:].rearrange("p (h c) -> p h c", h=H),
                        axis=AX.X, op=OP.add)
                    nc.scalar.activation(out=ex_all[:, c * H:(c + 1) * H],
                                         in_=al[:], func=ACTF.Exp)
                dps = pse.tile([128, H], F32, tag="eH")
                for c in range(K):
                    nc.tensor.matmul(out=dps[:], lhsT=mT[:, c * 128:(c + 1) * 128],
                                     rhs=ex_all[:, c * H:(c + 1) * H],
                                     start=(c == 0), stop=(c == K - 1))
                rdf = sb.tile([128, H], F32, tag="rdf")
                nc.vector.reciprocal(out=rdf[:], in_=dps[:])
                rden = sb.tile([128, H], BF16, tag="rden")
                nc.vector.tensor_copy(out=rden[:], in_=rdf[:])
                rde = pse.tile([128, K * H], F32, tag="eH")
                for c in range(K):
                    nc.tensor.matmul(out=rde[:, c * H:(c + 1) * H],
                                     lhsT=mde[:, c * 128:(c + 1) * 128],
                                     rhs=rden[:], start=True, stop=True)
                om = sb.tile([128, K * H], BF16, tag="om")
                nc.vector.tensor_tensor(out=om[:], in0=ex_all[:], in1=rde[:],
                                        op=OP.mult)
                wx = sbg.tile([128, K * HC], BF16, tag="wx")
                nc.vector.tensor_tensor(
                    out=wx[:].rearrange("p (k h c) -> p k h c", k=K, h=H),
                    in0=xlg[:].rearrange("p (k h c) -> p k h c", k=K, h=H),
                    in1=om[:].rearrange("p (k h o) -> p k h o", k=K, h=H, o=1)
                        .to_broadcast([128, K, H, C]),
                    op=OP.mult)
                hps = pse.tile([128, C], F32, tag="eH")
                n_mm = K * H
                i_mm = 0
                for c in range(K):
                    for h in range(H):
                        nc.tensor.matmul(
                            out=hps[:], lhsT=mT[:, c * 128:(c + 1) * 128],
                            rhs=wx[:, (c * H + h) * C:(c * H + h + 1) * C],
                            start=(i_mm == 0), stop=(i_mm == n_mm - 1))
                        i_mm += 1
                s2 = sb.tile([128, C], F32, tag="s2")
                nc.vector.scalar_tensor_tensor(
                    out=s2[:], in0=hps[:], scalar=0.25, in1=bw[f"biasrep{l}"][:],
                    op0=OP.mult, op1=OP.add)
                hout = sb.tile([128, C], BF16, tag="hout")
                if l < 2:
                    r = sb.tile([128, C], F32, tag="relu_t")
                    nc.scalar.activation(out=r[:], in_=s2[:], func=ACTF.Relu)
                    xm = sb.tile([128, C], F32, tag="xm_t")
                    nc.vector.tensor_scalar_min(out=xm[:], in0=s2[:], scalar1=0.0)
                    e = sb.tile([128, C], F32, tag="e_t")
                    nc.scalar.activation(out=e[:], in_=xm[:], func=ACTF.Exp)
                    nc.vector.scalar_tensor_tensor(
                        out=hout[:], in0=e[:], scalar=-1.0, in1=r[:],
                        op0=OP.add, op1=OP.add)
                    tps = ps.tile([128, 128], BF16, **MM)
                    nc.tensor.transpose(out=tps[:], in_=hout[:], identity=ident[:])
                    nc.any.tensor_copy(out=hT_dst[:, tl * 128:(tl + 1) * 128],
                                       in_=tps[:])
                else:
                    nc.vector.tensor_copy(out=hout[:], in_=s2[:])
                    gm = sb.tile([128, B], BF16, tag="gm")
                    nc.sync.dma_start(out=gm[:],
                                      in_=gmask_in[:, tl * B:(tl + 1) * B])
                    nc.tensor.matmul(out=pool_psum[:], lhsT=gm[:], rhs=hout[:],
                                     start=(tl == 0), stop=(tl == T - 1))
                if DEBUG and l == 0 and tl == 0:
                    dh = sb.tile([128, C], F32, tag="dbgh")
                    nc.vector.tensor_copy(out=dh[:], in_=hout[:])
                    nc.sync.dma_start(out=dbg["h1"][:], in_=dh[:])
                choff += K

        # =========================== Transformer =============================
        def ln_stats(src_bf, pidx):
            pt = psl.tile([1, L], F32, tag="acc2")
            nc.tensor.matmul(out=pt[:], lhsT=ones128b[:], rhs=src_bf,
                             start=True, stop=True)
            stg = sbt.tile([1, L], F32, tag="st_stage")
            nc.scalar.activation(out=stg[:], in_=pt[:], func=ACTF.Copy)
            nc.sync.dma_start(out=S_sb[pidx:pidx + 1, :], in_=stg[:])
            sq = sbt.tile([128, L], BF16, tag="sq_sb")
            nc.scalar.activation(out=sq[:], in_=src_bf, func=ACTF.Square)
            pt2 = psl.tile([1, L], F32, tag="acc2")
            nc.tensor.matmul(out=pt2[:], lhsT=ones128b[:], rhs=sq[:],
                             start=True, stop=True)
            stg2 = sbt.tile([1, L], F32, tag="st_stage2")
            nc.scalar.activation(out=stg2[:], in_=pt2[:], func=ACTF.Copy)
            nc.sync.dma_start(out=Q_sb[pidx:pidx + 1, :], in_=stg2[:])

        def ln_scalars():
            ac = sb2.tile([LP, 2 * L], BF16, tag="ln_ac")
            mu = sbt.tile([LP, L], F32, tag="ln_mu")
            nc.vector.tensor_scalar_mul(out=mu[:], in0=S_sb[:], scalar1=1.0 / D)
            var = sbt.tile([LP, L], F32, tag="ln_var")
            nc.vector.scalar_tensor_tensor(out=var[:], in0=mu[:], scalar=-1.0,
                                           in1=mu[:], op0=OP.mult, op1=OP.mult)
            nc.vector.scalar_tensor_tensor(out=var[:], in0=Q_sb[:], scalar=1.0 / D,
                                           in1=var[:], op0=OP.mult, op1=OP.add)
            sd = sbt.tile([LP, L], F32, tag="ln_sd")
            nc.scalar.activation(out=sd[:], in_=var[:], func=ACTF.Sqrt,
                                 bias=eps_col[:])
            rs = sbt.tile([LP, L], F32, tag="ln_rs")
            nc.vector.reciprocal(out=rs[:], in_=sd[:])
            nc.vector.tensor_copy(out=ac[:, :L], in_=rs[:])
            nc.vector.scalar_tensor_tensor(out=ac[:, L:], in0=mu[:], scalar=-1.0,
                                           in1=rs[:], op0=OP.mult, op1=OP.mult)
            return ac

        def ln_apply(r_bf, ac, pidx, g, b_, outbuf):
            rep2 = psl.tile([128, 2 * L], F32, tag="rep2")
            nc.tensor.matmul(out=rep2[:, :L],
                             lhsT=lnsel[:, pidx * 128:(pidx + 1) * 128],
                             rhs=ac[:, :L], start=True, stop=True)
            nc.tensor.matmul(out=rep2[:, L:],
                             lhsT=lnsel[:, pidx * 128:(pidx + 1) * 128],
                             rhs=ac[:, L:], start=True, stop=True)
            t1 = sbt.tile([128, L], F32, tag="ln_t1")
            nc.vector.tensor_tensor(out=t1[:], in0=r_bf, in1=rep2[:, :L],
                                    op=OP.mult)
            t2 = sbt.tile([128, L], F32, tag="ln_t2")
            nc.vector.tensor_tensor(out=t2[:], in0=t1[:], in1=rep2[:, L:],
                                    op=OP.add)
            nc.vector.tensor_scalar(out=outbuf, in0=t2[:],
                                    scalar1=fb(g), scalar2=fb(b_),
                                    op0=OP.mult, op1=OP.add)

        def tf_attn(l, p_):
            xT = xTs[:, p_ * L:(p_ + 1) * L]
            qh, kh = [], []
            for j in range(2):
                pt = ps.tile([128, L], F32, **MM)
                nc.tensor.matmul(out=pt[:],
                                 lhsT=bw[f"wqkvT{l}"][:, j * D:(j + 1) * D],
                                 rhs=xT, start=True, stop=True)
                for h in range(H):
                    st = sbt.tile([32, L], BF16, tag=f"qk_sb{j}{h}")
                    nc.scalar.activation(
                        out=st[:], in_=pt[h * 32:(h + 1) * 32, :],
                        func=ACTF.Identity,
                        bias=fb(tw[f"bqkv{l}"][h * 32:(h + 1) * 32, j:j + 1]))
                    (qh if j == 0 else kh).append(st)
            vn = sbt.tile([128, 4 * D], BF16, tag="vn")
            for i in range(4):
                pt = ps.tile([128, D], F32, **MM)
                nc.tensor.matmul(out=pt[:], lhsT=xT[:, i * 128:(i + 1) * 128],
                                 rhs=bw[f"wqkvT{l}"][:, 2 * D:3 * D],
                                 start=True, stop=False)
                nc.tensor.matmul(out=pt[:], lhsT=ones1[:],
                                 rhs=bw[f"bvrow{l}"][:], start=False, stop=True)
                nc.scalar.activation(out=vn[:, i * D:(i + 1) * D], in_=pt[:],
                                     func=ACTF.Copy)
            oT = sbt.tile([128, L], BF16, tag="oT")
            for h in range(H):
                exs = []
                for i in range(4):
                    pt = ps.tile([128, L], F32, **MM)
                    nc.tensor.matmul(out=pt[:], lhsT=kh[h][:, i * 128:(i + 1) * 128],
                                     rhs=qh[h][:], start=True, stop=True)
                    ext = sbt.tile([128, L], BF16, tag=f"ex_sb{i}")
                    nc.scalar.activation(out=ext[:], in_=pt[:], func=ACTF.Exp,
                                         scale=float(1.0 / np.sqrt(32.0)))
                    exs.append(ext)
                do_ps = psl.tile([33, L], F32, tag="acc2")
                for i in range(4):
                    nc.tensor.matmul(out=do_ps[32:33, :], lhsT=ones128b[:],
                                     rhs=exs[i][:], start=(i == 0), stop=(i == 3))
                    nc.tensor.matmul(out=do_ps[0:32, :],
                                     lhsT=vn[:, i * D + h * 32:i * D + (h + 1) * 32],
                                     rhs=exs[i][:], start=(i == 0), stop=(i == 3))
                rd = sbt.tile([1, L], BF16, tag="rd_sb")
                nc.vector.reciprocal(out=rd[:], in_=do_ps[32:33, :])
                rps = ps.tile([32, L], F32, **MM)
                nc.tensor.matmul(out=rps[:], lhsT=ones1[:, :32], rhs=rd[:],
                                 start=True, stop=True)
                rsb = sbt.tile([32, L], F32, tag="rd_rep")
                nc.scalar.activation(out=rsb[:], in_=rps[:], func=ACTF.Copy)
                nc.vector.tensor_tensor(out=oT[h * 32:(h + 1) * 32, :],
                                        in0=do_ps[0:32, :], in1=rsb[:], op=OP.mult)
            apt = ps.tile([128, L], F32, **MM)
            nc.tensor.matmul(out=apt[:], lhsT=bw[f"woT{l}"][:], rhs=oT[:],
                             start=True, stop=True)
            ao = sbt.tile([128, L], F32, tag="ao_sb")
            nc.scalar.activation(out=ao[:], in_=apt[:], func=ACTF.Identity,
                                 bias=fb(tw[f"bo{l}"][:]))
            nc.vector.tensor_tensor(out=xT, in0=xT, in1=ao[:], op=OP.add)
            return xT

        def tf_ff(l, xln, p_):
            h1s = []
            for f in range(NF):
                pt = ps.tile([128, L], F32, **MM)
                nc.tensor.matmul(out=pt[:], lhsT=bw[f"w1T{l}"][:, f * D:(f + 1) * D],
                                 rhs=xln, start=True, stop=True)
                hsb = sbt.tile([128, L], BF16, tag=f"ff1_sb{f % 4}")
                nc.scalar.activation(out=hsb[:], in_=pt[:], func=ACTF.Relu,
                                     bias=fb(tw[f"b1{l}"][:, f:f + 1]))
                h1s.append(hsb)
            pt2 = psl.tile([128, L], F32, tag="acc2")
            for f in range(NF):
                nc.tensor.matmul(out=pt2[:], lhsT=bw[f"w2T{l}"][:, f * D:(f + 1) * D],
                                 rhs=h1s[f][:], start=(f == 0), stop=(f == NF - 1))
            ffo = sbt.tile([128, L], F32, tag="ffo")
            nc.scalar.activation(out=ffo[:], in_=pt2[:], func=ACTF.Identity,
                                 bias=fb(tw[f"b2{l}"][:]))
            out = xTs[:, p_ * L:(p_ + 1) * L]
            nc.vector.tensor_tensor(out=out, in0=xln, in1=ffo[:], op=OP.add)
            return out

        # ---- emit ----
        for p_ in range(LP):
            oh_sb = sbt.tile([VP, L], F32R, tag="oh_sb")
            nc.sync.dma_start(out=oh_sb[:],
                              in_=onehotT[:, p_ * L:(p_ + 1) * L].bitcast(F32R))
            for j in range(4):
                pt = ps.tile([128, 128], F32, **MM)
                nc.tensor.matmul(out=pt[:], lhsT=tw["emb_pad"][:],
                                 rhs=oh_sb[:, j * 128:(j + 1) * 128],
                                 start=True, stop=True)
                nc.scalar.activation(
                    out=xTs[:, p_ * L + j * 128:p_ * L + (j + 1) * 128],
                    in_=pt[:], func=ACTF.Copy)

        node_stage(0, xT0_sb[:], 80)
        barrier(0)
        pool_ps = psl.tile([B, C], F32, tag="pool")
        edge_stage(0, hT_a[:], pool_ps)

        for l in range(NL):
            for p_ in range(LP):
                r1 = tf_attn(l, p_)
                ln_stats(r1, p_)
            ac = ln_scalars()
            for p_ in range(LP):
                r1 = xTs[:, p_ * L:(p_ + 1) * L]
                xln = sb2.tile([128, L], BF16, tag=f"xln{p_ % 2}")
                ln_apply(r1, ac, p_, tw[f"ln1g{l}"][:, 0:1],
                         tw[f"ln1b{l}"][:, 0:1], xln[:])
                r2 = tf_ff(l, xln[:], p_)
                ln_stats(r2, p_)
            ac2 = ln_scalars()
            for p_ in range(LP):
                r2 = xTs[:, p_ * L:(p_ + 1) * L]
                ln_apply(r2, ac2, p_, tw[f"ln2g{l}"][:, 0:1],
                         tw[f"ln2b{l}"][:, 0:1], r2)
            if l == 0:
                node_stage(1, hT_a[:], 128)
                barrier(1)
                edge_stage(1, hT_b[:], pool_ps)

        node_stage(2, hT_b[:], 128)
        barrier(2)
        edge_stage(2, hT_a[:], pool_ps)

        # p_emb
        pm = sb2.tile([128, LP], F32R, tag="pm")
        for p_ in range(LP):
            nc.vector.tensor_reduce(out=pm[:, p_:p_ + 1],
                                    in_=xTs[:, p_ * L:(p_ + 1) * L],
                                    axis=AX.X, op=OP.add)
        pept = psl.tile([128, LP], F32, tag="acc2")
        nc.tensor.matmul(out=pept[:], lhsT=tw["fcwT"][:], rhs=pm[:],
                         start=True, stop=True)
        pemb = sb2.tile([128, LP], F32R, tag="pemb")
        nc.scalar.activation(out=pemb[:], in_=pept[:], func=ACTF.Identity,
                             bias=fb(tw["fcb"][:]))
        if DEBUG:
            dpe = sb.tile([128, LP], F32, tag="dbgpe")
            nc.vector.tensor_copy(out=dpe[:], in_=pemb[:])
            nc.sync.dma_start(out=dbg["pemb"][:], in_=dpe[:])

        dsum = sb2.tile([B, C], F32, tag="dsum")
        nc.scalar.activation(out=dsum[:], in_=pool_ps[:], func=ACTF.Copy)
        nc.sync.dma_start(out=dpool_loc[:], in_=dsum[:])
        nc.gpsimd.collective_compute(
            "AllReduce", OP.add, ins=[dpool_loc[:]], outs=[dpool_sh[:]],
            replica_groups=[list(range(NCORES))])
        dall = sb2.tile([B, C], F32, tag="dall")
        nc.sync.dma_start(out=dall[:], in_=dpool_sh[:])
        demb = sb2.tile([B, C], F32R, tag="demb")
        nc.vector.tensor_scalar_mul(out=demb[:], in0=dall[:], scalar1=invc[:])
        if DEBUG:
            dde = sb.tile([B, C], F32, tag="dbgde")
            nc.vector.tensor_copy(out=dde[:], in_=demb[:])
            nc.sync.dma_start(out=dbg["demb"][:], in_=dde[:])

        dpt = psl.tile([C, LP], F32, tag="acc2")
        nc.tensor.matmul(out=dpt[:], lhsT=demb[:], rhs=selsb[:],
                         start=True, stop=True)
        dT = sb2.tile([C, LP], F32R, tag="dT")
        nc.scalar.activation(out=dT[:], in_=dpt[:], func=ACTF.Copy)

        z1 = []
        for j in range(4):
            pt = ps.tile([128, LP], F32, **MM)
            nc.tensor.matmul(out=pt[:], lhsT=tw["r1wT"][:, j * 128:(j + 1) * 128],
                             rhs=dT[:], start=True, stop=False)
            nc.tensor.matmul(out=pt[:],
                             lhsT=tw["r1wT"][:, 512 + j * 128:512 + (j + 1) * 128],
                             rhs=pemb[:], start=False, stop=True)
            zz = sbt.tile([128, LP], F32R, tag=f"z1_{j}")
            nc.scalar.activation(out=zz[:], in_=pt[:], func=ACTF.Relu,
                                 bias=fb(tw["r1b"][:, j:j + 1]))
            z1.append(zz)
        pt = psl.tile([128, LP], F32, tag="acc2")
        for j in range(4):
            nc.tensor.matmul(out=pt[:], lhsT=tw["r2wT"][:, j * 128:(j + 1) * 128],
                             rhs=z1[j][:], start=(j == 0), stop=(j == 3))
        z2 = sb2.tile([128, LP], F32R, tag="z2")
        nc.scalar.activation(out=z2[:], in_=pt[:], func=ACTF.Relu,
                             bias=fb(tw["r2b"][:]))
        pt3 = psl.tile([1, LP], F32, tag="acc2")
        nc.tensor.matmul(out=pt3[:], lhsT=tw["r3wT"][:], rhs=z2[:],
                         start=True, stop=True)
        pred = sb2.tile([1, LP], F32, tag="pred")
        nc.scalar.activation(out=pred[:], in_=pt3[:], func=ACTF.Identity,
                             bias=fb(tw["r3b"][:]))
        nc.sync.dma_start(out=predT_o[:], in_=pred[:])

        for w1k, b1k, w2k, b2k, xin, outT in (
                ("dp1T", "dp1b", "dp2T", "dp2b", dT, zdT_o),
                ("pp1T", "pp1b", "pp2T", "pp2b", pemb, zpT_o)):
            pt = psl.tile([64, LP], F32, tag="acc2")
            nc.tensor.matmul(out=pt[:], lhsT=tw[w1k][:], rhs=xin[:],
                             start=True, stop=True)
            t1 = sbt.tile([64, LP], F32R, tag="hd_t1")
            nc.scalar.activation(out=t1[:], in_=pt[:], func=ACTF.Relu,
                                 bias=fb(tw[b1k][:]))
            pt2 = psl.tile([64, LP], F32, tag="acc2")
            nc.tensor.matmul(out=pt2[:], lhsT=tw[w2k][:], rhs=t1[:],
                             start=True, stop=True)
            t2 = sbt.tile([64, LP], F32, tag="hd_t2")
            nc.scalar.activation(out=t2[:], in_=pt2[:], func=ACTF.Identity,
                                 bias=fb(tw[b2k][:]))
            nc.sync.dma_start(out=outT[:], in_=t2[:])

        for p in reversed(ctxs):
            p.__exit__(None, None, None)

    nc.compile()
    return nc


_CACHE = {}


def kernel(x, edge_index, batch, target, params):
    in_maps, meta, CH = _prep(np.asarray(x), np.asarray(edge_index),
                              np.asarray(batch), np.asarray(target), params)
    if "prog" not in _CACHE:
        _CACHE["prog"] = _build(meta, CH)
    nc = _CACHE["prog"]
    res = run_bass_kernel_spmd(nc, in_maps, list(range(NCORES)))
    pred = np.concatenate([res.results[k]["predT"].T for k in range(NCORES)], 0)
    zd = np.concatenate([res.results[k]["zdT"].T for k in range(NCORES)], 0)
    zp = np.concatenate([res.results[k]["zpT"].T for k in range(NCORES)], 0)
    kernel._res = res
    return (np.ascontiguousarray(pred, dtype=np.float32),
            np.ascontiguousarray(zd, dtype=np.float32),
            np.ascontiguousarray(zp, dtype=np.float32))
